# revision 24
# baseline (speedup 1.0000x reference)
"""DynamicSparseMoE grouped-GEMM kernel for 8 TRN2 NeuronCores.

out[t] = tokens[t] @ weight[exp_ids[t]]   (T=8192, E=8, D=2048 -> 2048)

Strategy (expert-parallel with quarter-granularity overflow rebalance):
  - Host sorts tokens by expert. Core e owns expert e's weight and its
    first 1024 routed tokens (8 full 128-token blocks, zero-padded).
    Tokens beyond 1024 ("overflow") are shipped in <=32-token chunks to
    other cores' B-units, so no core pays for a 9th 128-block: 528 PE
    slot-times/core (512 + 16 for the B-unit) vs 576 unbalanced.
  - Inputs cast to fp16 on host (PE 1 cyc/row vs fp32's 4; PSUM
    accumulates fp32; rel-err ~3.6e-4). Stationary operand is a token
    block x[d-block, 128t]; moving operand is a 512-wide weight slice;
    PSUM gets out[t-block, o-slice] in natural orientation. Warm slot
    rate measured 216 ns = 512cols/2.4GHz + NX overhead (the floor);
    LDWEIGHTS is hidden even when the stationary changes every 2 MMs.
  - Unit order: quadA (blocks 0-3 x os{0,1}, kb-major), quadB (os{2,3}),
    pair(4,5), single(6), single(7), B-unit. The quad os-phasing halves
    startup weight demand; singles stream each block's 1 MB output
    while the next computes; the B-unit (32 tokens x 4 os slices as
    FOUR concurrent 32-wide column-group matmuls, tile_position=(0,32g),
    separate PSUM banks) drains just 2x64 KB in the tail.
  - DMA: the 16 DMA engines round-robin ALL in-flight transfers at
    ~21 GB/s each, and a ring kick costs ~0.63 us of engine time — so
    the startup splits the first tiles into 32-64 KB micro-kicks
    alternating Sync/Scalar rings (doubles kick rate, multiplies
    in-flight lanes); the bulk stream then rides Sync in consumption
    order: x/wA interleaved per kb, wB, late-x, xq, wq. Out-kicks ride
    Sync AFTER all input kicks (program order) so they never gate a
    cast; the final B-unit kicks use both rings.
  - PSUM->SBUF evacuation casts alternate Vector/Scalar (each ~0.7 us;
    those queues carry nothing else, so the 8-bank turnover between
    units is covered by the stop-stagger within a unit's last kb).
    ~45 warm-up matmuls on a memset tile bridge first-data latency and
    open the HAM clock gate before real matmuls begin.
"""

import os

import numpy as np

# A previously wedged NeuronCore (NRT_EXEC_UNIT_UNRECOVERABLE) recovers on
# the next init when core reset is requested; must be set before NRT init.
os.environ.setdefault("NEURON_RT_RESET_CORES", "1")

P = 128
D = 2048
E = 8
KB = D // P  # 16 contraction blocks
NOS = 4  # 4 moving slices of 512 over the 2048 output dim
NS = D // NOS  # 512
HD = D // 2  # 1024
QW = 32  # B-unit token width (one column-group quarter)

_cache = {}


def _ensure_imports():
    try:
        import concourse.bass  # noqa: F401
    except ImportError:
        import sys

        for p in ("/opt/trn_rl_repo", "/opt/pypackages"):
            if p not in sys.path:
                sys.path.append(p)


def _np_dt(compute_dt):
    if compute_dt == "float16":
        return np.float16
    import ml_dtypes

    return ml_dtypes.bfloat16


def _build(n_full, tail, compute_dt="float16"):
    """Build + compile the per-core Bass program.

    n_full full 128-token blocks of the A-expert; tail is "quarter"
    (32-token B-unit of a second expert), "packed" (64-token half-block
    of the A-expert), or "none".
    """
    _ensure_imports()
    import concourse.bacc as bacc
    import concourse.mybir as mybir
    import concourse.tile as tile

    cdt = getattr(mybir.dt, compute_dt)
    assert n_full >= 6
    quarter = tail == "quarter"
    packed = tail == "packed"
    NA = 4 * P  # x columns/kb for the quad (blocks 0-3)
    NC = 2 * P  # x columns/kb for the pair (blocks 4-5)
    ND = (n_full - 6) * P + (64 if packed else 0)  # blocks 6.. + packed tail
    out_rows = n_full * P + (64 if packed else QW if quarter else 0)

    nc = bacc.Bacc(None, target_bir_lowering=False, debug=False)
    # x is packed kb-major on the host so one DMA covers a GROUP of kb
    # blocks with >=2 KB per-partition rows (1 KB rows run ~280 GB/s),
    # and split by consuming unit so each piece can be kicked with wide
    # arrival margin (late-stream transfer latency is noisy by tens of
    # us under the per-queue FIFO + fair-share DMA engines).
    xA_d = nc.declare_dram_parameter("xA", [P, KB * NA], cdt, isOutput=False)
    xC_d = nc.declare_dram_parameter("xC", [P, KB * NC], cdt, isOutput=False)
    xD_d = nc.declare_dram_parameter("xD", [P, KB * ND], cdt, isOutput=False)
    w_d = nc.declare_dram_parameter("w", [D, D], cdt, isOutput=False)
    if quarter:
        xq_d = nc.declare_dram_parameter("xq", [P, KB * QW], cdt, isOutput=False)
        wq_d = nc.declare_dram_parameter("wq", [D, D], cdt, isOutput=False)
        wq_t = wq_d.rearrange("(k p) o -> p k o", p=P)
    out_d = nc.declare_dram_parameter("out", [out_rows, D], cdt, isOutput=True)

    w_t = w_d.rearrange("(k p) o -> p k o", p=P)  # [128, 16, 2048]

    with tile.TileContext(nc) as tc:
        with (
            tc.tile_pool(name="wp", bufs=1) as wp,
            tc.tile_pool(name="xp", bufs=1) as xp,
            # 24 staging tiles (3 MB): enough for quadA+quadB+pair slices so
            # a cast never waits on an out-DMA completing — out-kicks sit
            # behind ALL input kicks on Sync, which drain late on the run's
            # slowest core (op starvation -> cast stall -> PSUM-bank stall).
            tc.tile_pool(name="op", bufs=24) as op,
            tc.tile_pool(name="pp", bufs=8, space="PSUM") as pp,
        ):
            xa2 = [
                xp.tile([P, 2 * NA], cdt, tag=f"xa2_{j}", name=f"xa2_{j}")
                for j in range(KB // 2)
            ]
            wA = [
                wp.tile([P, HD], cdt, tag=f"wA{kb}", name=f"wA{kb}")
                for kb in range(KB)
            ]
            wB = [
                wp.tile([P, HD], cdt, tag=f"wB{kb}", name=f"wB{kb}")
                for kb in range(KB)
            ]

            # --- startup micro-kicks, alternating rings ---
            # Each in-flight transfer progresses at ~21 GB/s (fair share of
            # the 16 DMA engines), so the first tiles are split into small
            # parallel kicks in exact consumption order.
            x2 = 2 * NA
            early = [
                (nc.sync, xa2[0][:, :P], xA_d[:, :P]),
                (nc.scalar, wA[0][:, : NS // 2], w_t[:, 0, : NS // 2]),
                (nc.sync, wA[0][:, NS // 2 : NS], w_t[:, 0, NS // 2 : NS]),
                (nc.scalar, xa2[0][:, P : P + 192], xA_d[:, P : P + 192]),
                (nc.sync, xa2[0][:, P + 192 : NA], xA_d[:, P + 192 : NA]),
                (nc.scalar, wA[0][:, NS : NS + 256], w_t[:, 0, NS : NS + 256]),
                (nc.sync, wA[0][:, NS + 256 :], w_t[:, 0, NS + 256 : HD]),
                (nc.scalar, xa2[0][:, NA : NA + 256], xA_d[:, NA : NA + 256]),
                (nc.sync, xa2[0][:, NA + 256 :], xA_d[:, NA + 256 : x2]),
                (nc.scalar, wA[1][:, :NS], w_t[:, 1, :NS]),
                (nc.sync, wA[1][:, NS:], w_t[:, 1, NS:HD]),
                (nc.scalar, xa2[1][:, :NA], xA_d[:, x2 : x2 + NA]),
                (nc.sync, xa2[1][:, NA:], xA_d[:, x2 + NA : 2 * x2]),
                (nc.scalar, wA[2][:, :NS], w_t[:, 2, :NS]),
                (nc.sync, wA[2][:, NS:], w_t[:, 2, NS:HD]),
                (nc.scalar, wA[3][:, :NS], w_t[:, 3, :NS]),
                (nc.sync, wA[3][:, NS:], w_t[:, 3, NS:HD]),
            ]
            for ring, dst, src in early:
                ring.dma_start(dst, src)
            # bulk stream on Sync in consumption order
            for j in range(2, KB // 2):
                nc.sync.dma_start(xa2[j][:], xA_d[:, j * x2 : (j + 1) * x2])
                nc.sync.dma_start(wA[2 * j][:], w_t[:, 2 * j, :HD])
                nc.sync.dma_start(wA[2 * j + 1][:], w_t[:, 2 * j + 1, :HD])
            # pair-x (needed ~64us) rides right behind the wA stream and
            # ahead of most of wB; blocks-6+ x interleaves 1:1 with wB.
            # Everything late-stream gets >=4us (mostly >=30us) of margin.
            xc4 = [
                xp.tile([P, 4 * NC], cdt, tag=f"xc4_{j}", name=f"xc4_{j}")
                for j in range(KB // 4)
            ]
            xd4 = [
                xp.tile([P, 4 * ND], cdt, tag=f"xd4_{j}", name=f"xd4_{j}")
                for j in range(KB // 4)
            ]
            for j in range(KB // 4):
                nc.sync.dma_start(wB[2 * j][:], w_t[:, 2 * j, HD:])
                nc.sync.dma_start(
                    xc4[j][:], xC_d[:, j * 4 * NC : (j + 1) * 4 * NC]
                )
                nc.sync.dma_start(wB[2 * j + 1][:], w_t[:, 2 * j + 1, HD:])
            for j in range(KB // 4):
                nc.sync.dma_start(wB[KB // 2 + 2 * j][:], w_t[:, KB // 2 + 2 * j, HD:])
                nc.sync.dma_start(
                    xd4[j][:], xD_d[:, j * 4 * ND : (j + 1) * 4 * ND]
                )
                nc.sync.dma_start(
                    wB[KB // 2 + 2 * j + 1][:], w_t[:, KB // 2 + 2 * j + 1, HD:]
                )
            if quarter:
                xq_sb = xp.tile([P, KB * QW], cdt, tag="xq", name="xq")
                nc.sync.dma_start(xq_sb[:], xq_d[:])
                wq = []
                for kb in range(KB):
                    wk = wp.tile([P, D], cdt, tag=f"wq{kb}", name=f"wq{kb}")
                    nc.sync.dma_start(wk[:], wq_t[:, kb, :])
                    wq.append(wk)

            def w_slice(kb, osl):
                w = wA[kb] if osl < 2 else wB[kb]
                s = (osl % 2) * NS
                return w[:, s : s + NS]

            def lhs(kb, b):
                if b < 4:
                    off = (kb % 2) * NA + b * P
                    return xa2[kb // 2][:, off : off + P]
                if b < 6:
                    off = (kb % 4) * NC + (b - 4) * P
                    return xc4[kb // 4][:, off : off + P]
                off = (kb % 4) * ND + (b - 6) * P
                return xd4[kb // 4][:, off : off + P]

            def lhs_packed(kb):
                off = (kb % 4) * ND + (n_full - 6) * P
                return xd4[kb // 4][:, off : off + 64]

            # --- PE pre-warm: HAM clock-gates the PE at 1.2 GHz until
            # ~3.4us of sustained activity; dummy matmuls bridge first-data
            # DMA latency so real matmuls run warm.
            warm = xp.tile([P, 64], cdt, tag="warm")
            nc.vector.memset(warm[:], 0.0)
            pw = pp.tile([P, NS], mybir.dt.float32, tag="ps", name="ps_warm")
            # sized so real matmuls start ~12.5us with ~3 kb-groups of data
            # buffered and the clock already at 2.4 GHz — the DMA ramp can't
            # feed a 1.7us/kb warm pace before that, and an early data stall
            # resets the HAM activity window (cold cascade on slow cores)
            for _ in range(150):
                nc.tensor.matmul(
                    pw[:64, :64],
                    lhsT=warm[:, :64],
                    rhs=warm[:, :64],
                    start=True,
                    stop=True,
                )

            cast_idx = [0]

            def evac(slices, rings=(nc.sync,)):
                # psum -> sbuf (fp32 -> cdt cast) on alternating Vector /
                # Scalar queues — casts free PSUM banks for the next unit,
                # so those queues carry nothing else. Casts are emitted
                # before any out-kick; kicks ride the given ring(s), where
                # (in program order) they sit behind the input kicks and
                # drain once those finish.
                kicks = []
                for ps_ap, rows, r0, osl in slices:
                    o_sb = op.tile([P, NS], cdt, tag="o", name=f"o_{r0}_{osl}")
                    if cast_idx[0] % 2 == 0:
                        nc.vector.tensor_copy(o_sb[rows, :], ps_ap)
                    else:
                        nc.scalar.copy(o_sb[rows, :], ps_ap)
                    cast_idx[0] += 1
                    kicks.append((o_sb, rows, r0, osl))
                for i, (o_sb, rows, r0, osl) in enumerate(kicks):
                    nrows = (rows.stop if rows.stop is not None else P) - (
                        rows.start or 0
                    )
                    rings[i % len(rings)].dma_start(
                        out_d[r0 : r0 + nrows, osl * NS : (osl + 1) * NS],
                        o_sb[rows, :],
                    )

            full = slice(0, P)

            def run_group(blocks, osls):
                # kb-major accumulation over the given (block, os) banks
                ps = {
                    (b, osl): pp.tile(
                        [P, NS], mybir.dt.float32, tag="ps", name=f"ps_{b}_{osl}"
                    )
                    for b in blocks
                    for osl in osls
                }
                for kb in range(KB):
                    for b in blocks:
                        for osl in osls:
                            nc.tensor.matmul(
                                ps[(b, osl)][:],
                                lhsT=lhs(kb, b),
                                rhs=w_slice(kb, osl),
                                start=(kb == 0),
                                stop=(kb == KB - 1),
                            )
                evac(
                    [(ps[(b, o)][:], full, b * P, o) for b in blocks for o in osls]
                )

            def run_packed(os_pair, rings=(nc.sync,)):
                # 64-wide half-block: the two os slices run CONCURRENTLY in
                # the PE's column-group halves (tile_position auto-derived
                # from PSUM base partition 0/64; separate banks).
                ps = {
                    osl: pp.tile(
                        [P, NS], mybir.dt.float32, tag="ps", name=f"ps_pk{osl}"
                    )
                    for osl in os_pair
                }
                for kb in range(KB):
                    for osl in os_pair:
                        dst = ps[osl][:64, :] if osl % 2 == 0 else ps[osl][64:, :]
                        nc.tensor.matmul(
                            dst,
                            lhsT=lhs_packed(kb),
                            rhs=w_slice(kb, osl),
                            start=(kb == 0),
                            stop=(kb == KB - 1),
                        )
                slices = []
                for osl in os_pair:
                    rows = slice(0, 64) if osl % 2 == 0 else slice(64, P)
                    slices.append((ps[osl][rows, :], rows, n_full * P, osl))
                evac(slices, rings=rings)

            def run_bunit():
                # 32 overflow tokens of a second expert: all four os slices
                # run CONCURRENTLY in the PE's four 32-wide column groups
                # (explicit tile_position; one PSUM bank per group).
                ps = [
                    pp.tile([P, NS], mybir.dt.float32, tag="ps", name=f"ps_q{g}")
                    for g in range(4)
                ]
                for kb in range(KB):
                    for g in range(4):
                        nc.tensor.matmul(
                            ps[g][g * QW : (g + 1) * QW, :],
                            lhsT=xq_sb[:, kb * QW : (kb + 1) * QW],
                            rhs=wq[kb][:, g * NS : (g + 1) * NS],
                            start=(kb == 0),
                            stop=(kb == KB - 1),
                            tile_position=(0, g * QW),
                        )
                slices = []
                for g in range(4):
                    rows = slice(g * QW, (g + 1) * QW)
                    slices.append((ps[g][rows, :], rows, n_full * P, g))
                evac(slices, rings=(nc.sync, nc.scalar))

            # --- unit schedule ---
            run_group([0, 1, 2, 3], (0, 1))  # quadA: rides the startup stream
            run_group([0, 1, 2, 3], (2, 3))  # quadB
            mids = list(range(4, n_full - 2))
            while len(mids) >= 2:
                run_group(mids[:2], (0, 1, 2, 3))
                mids = mids[2:]
            if mids:
                run_group(mids, (0, 1, 2, 3))
            if packed:
                run_packed((0, 1))
            run_group([n_full - 2], (0, 1, 2, 3))
            run_group([n_full - 1], (0, 1, 2, 3))
            if packed:
                run_packed((2, 3), rings=(nc.sync, nc.scalar))
            if quarter:
                run_bunit()
    nc.compile()
    return nc


def _get_nc(n_full, tail, compute_dt):
    key = (n_full, tail, compute_dt)
    if key not in _cache:
        _cache[key] = _build(n_full, tail, compute_dt)
    return _cache[key]


def _pack_x(xt):
    # kb-major packs: xA[p, kb*NA + t] = xt[kb*128+p, t] for the quad's
    # columns (blocks 0-3), xC for blocks 4-5, xD for blocks 6.. + tail
    C = xt.shape[1]
    NA, NC = 4 * P, 2 * P
    x3 = xt.reshape(KB, P, C)

    def pack(lo, hi):
        return np.ascontiguousarray(
            x3[:, :, lo:hi].transpose(1, 0, 2).reshape(P, KB * (hi - lo))
        )

    return pack(0, NA), pack(NA, NA + NC), pack(NA + NC, C)


def kernel(tokens, weight, exp_ids, _trace=False, _compute_dt="float16"):
    _ensure_imports()
    from concourse.bass_utils import run_bass_kernel_spmd

    tokens = np.asarray(tokens)
    weight = np.asarray(weight)
    exp_ids = np.asarray(exp_ids)
    T = tokens.shape[0]

    order = np.argsort(exp_ids, kind="stable")
    counts = np.bincount(exp_ids, minlength=E)
    starts = np.zeros(E + 1, dtype=np.int64)
    np.cumsum(counts, out=starts[1:])

    npdt = _np_dt(_compute_dt)
    tokens_c = tokens.astype(npdt)
    weight_c = weight.astype(npdt)

    CAP_A = 8 * P  # per-core full-block capacity
    # overflow chunks: expert, start within expert's sorted run, length
    chunks = []
    for e in range(E):
        over = int(counts[e]) - CAP_A
        s = CAP_A
        while over > 0:
            n = min(QW, over)
            chunks.append((e, s, n))
            s += n
            over -= n

    if 0 < len(chunks) <= E:
        # quarter-rebalanced path: core e gets expert e's first <=1024
        # tokens as 8 full blocks; chunk i rides core i's B-unit.
        n_full, tail = 8, "quarter"
        NA = 4 * P
        in_maps = []
        for c in range(E):
            na = min(int(counts[c]), CAP_A)
            idx = order[starts[c] : starts[c] + na]
            xt = np.zeros((D, CAP_A), dtype=npdt)
            xt[:, :na] = tokens_c[idx].T
            xA, xC, xD = _pack_x(xt)
            if c < len(chunks):
                e, s, n = chunks[c]
                idx_b = order[starts[e] + s : starts[e] + s + n]
                xqt = np.zeros((D, QW), dtype=npdt)
                xqt[:, :n] = tokens_c[idx_b].T
                wq = np.ascontiguousarray(weight_c[e])
            else:
                xqt = np.zeros((D, QW), dtype=npdt)
                wq = np.ascontiguousarray(weight_c[c])  # unused; avoids zeros
            xq = np.ascontiguousarray(
                xqt.reshape(KB, P, QW).transpose(1, 0, 2).reshape(P, KB * QW)
            )
            in_maps.append(
                {
                    "xA": xA,
                    "xC": xC,
                    "xD": xD,
                    "w": np.ascontiguousarray(weight_c[c]),
                    "xq": xq,
                    "wq": wq,
                }
            )
    else:
        # fallback: pure expert-parallel with per-core capacity from the
        # largest expert (packed 64-token half-block when the remainder fits)
        cap = max(int(counts.max()), 6 * P + 1)
        n_full = cap // P
        rem = cap - n_full * P
        if rem > 64:
            n_full += 1
            tail = "none"
        else:
            tail = "packed" if rem > 0 else "none"
        CX = n_full * P + (64 if tail == "packed" else 0)
        NA = 4 * P
        in_maps = []
        for e in range(E):
            idx = order[starts[e] : starts[e + 1]]
            xt = np.zeros((D, CX), dtype=npdt)
            xt[:, : counts[e]] = tokens_c[idx].T
            xA, xC, xD = _pack_x(xt)
            in_maps.append(
                {"xA": xA, "xC": xC, "xD": xD, "w": np.ascontiguousarray(weight_c[e])}
            )

    nc = _get_nc(n_full, tail, _compute_dt)
    res = run_bass_kernel_spmd(
        nc,
        in_maps,
        core_ids=list(range(E)),
        trace=_trace,
        trace_cores=list(range(E)) if _trace else None,
    )

    out = np.empty((T, D), dtype=np.float32)
    if tail == "quarter":
        for c in range(E):
            na = min(int(counts[c]), CAP_A)
            idx = order[starts[c] : starts[c] + na]
            out[idx] = res.results[c]["out"][:na, :].astype(np.float32)
            if c < len(chunks):
                e, s, n = chunks[c]
                idx_b = order[starts[e] + s : starts[e] + s + n]
                out[idx_b] = res.results[c]["out"][CAP_A : CAP_A + n, :].astype(
                    np.float32
                )
    else:
        for e in range(E):
            idx = order[starts[e] : starts[e + 1]]
            out[idx] = res.results[e]["out"][: counts[e], :].astype(np.float32)
    if _trace:
        return out, res
    return out


# revision 25
# speedup vs baseline: 1.0009x; 1.0009x over previous
"""DynamicSparseMoE grouped-GEMM kernel for 8 TRN2 NeuronCores.

out[t] = tokens[t] @ weight[exp_ids[t]]   (T=8192, E=8, D=2048 -> 2048)

Strategy (expert-parallel with quarter-granularity overflow rebalance):
  - Host sorts tokens by expert. Core e owns expert e's weight and its
    first 1024 routed tokens (8 full 128-token blocks, zero-padded).
    Tokens beyond 1024 ("overflow") are shipped in <=32-token chunks to
    other cores' B-units, so no core pays for a 9th 128-block: 528 PE
    slot-times/core (512 + 16 for the B-unit) vs 576 unbalanced.
  - Inputs cast to fp16 on host (PE 1 cyc/row vs fp32's 4; PSUM
    accumulates fp32; rel-err ~3.6e-4). Stationary operand is a token
    block x[d-block, 128t]; moving operand is a 512-wide weight slice;
    PSUM gets out[t-block, o-slice] in natural orientation. Warm slot
    rate measured 216 ns = 512cols/2.4GHz + NX overhead (the floor);
    LDWEIGHTS is hidden even when the stationary changes every 2 MMs.
  - Unit order: quadA (blocks 0-3 x os{0,1}, kb-major), quadB (os{2,3}),
    pair(4,5), single(6), single(7), B-unit. The quad os-phasing halves
    startup weight demand; singles stream each block's 1 MB output
    while the next computes; the B-unit (32 tokens x 4 os slices as
    FOUR concurrent 32-wide column-group matmuls, tile_position=(0,32g),
    separate PSUM banks) drains just 2x64 KB in the tail.
  - DMA: the 16 DMA engines round-robin ALL in-flight transfers at
    ~21 GB/s each, and a ring kick costs ~0.63 us of engine time — so
    the startup splits the first tiles into 32-64 KB micro-kicks
    alternating Sync/Scalar rings (doubles kick rate, multiplies
    in-flight lanes); the bulk stream then rides Sync in consumption
    order: x/wA interleaved per kb, wB, late-x, xq, wq. Out-kicks ride
    Sync AFTER all input kicks (program order) so they never gate a
    cast; the final B-unit kicks use both rings.
  - PSUM->SBUF evacuation casts alternate Vector/Scalar (each ~0.7 us;
    those queues carry nothing else, so the 8-bank turnover between
    units is covered by the stop-stagger within a unit's last kb). The
    24-tile output staging pool prevents cast-stalls when the slowest
    core's input stream drains late (out-kicks queue behind it).
    ~150 warm-up matmuls on a memset tile bridge the DMA ramp (real
    matmuls start ~12.5us warm with ~3 kb-groups buffered — an early
    data stall would reset the HAM activity window and cascade cold).
  - Measured: 134.9 us (baseline 140.3); floor ~= 7.2 preamble + 5.3
    warm-up/first-data + 113.9 slots + ~6 tail/teardown = ~132.5.
"""

import os

import numpy as np

# A previously wedged NeuronCore (NRT_EXEC_UNIT_UNRECOVERABLE) recovers on
# the next init when core reset is requested; must be set before NRT init.
os.environ.setdefault("NEURON_RT_RESET_CORES", "1")

P = 128
D = 2048
E = 8
KB = D // P  # 16 contraction blocks
NOS = 4  # 4 moving slices of 512 over the 2048 output dim
NS = D // NOS  # 512
HD = D // 2  # 1024
QW = 32  # B-unit token width (one column-group quarter)

_cache = {}


def _ensure_imports():
    try:
        import concourse.bass  # noqa: F401
    except ImportError:
        import sys

        for p in ("/opt/trn_rl_repo", "/opt/pypackages"):
            if p not in sys.path:
                sys.path.append(p)


def _np_dt(compute_dt):
    if compute_dt == "float16":
        return np.float16
    import ml_dtypes

    return ml_dtypes.bfloat16


def _build(n_full, tail, compute_dt="float16"):
    """Build + compile the per-core Bass program.

    n_full full 128-token blocks of the A-expert; tail is "quarter"
    (32-token B-unit of a second expert), "packed" (64-token half-block
    of the A-expert), or "none".
    """
    _ensure_imports()
    import concourse.bacc as bacc
    import concourse.mybir as mybir
    import concourse.tile as tile

    cdt = getattr(mybir.dt, compute_dt)
    assert n_full >= 6
    quarter = tail == "quarter"
    packed = tail == "packed"
    NA = 4 * P  # x columns/kb for the quad (blocks 0-3)
    NC = 2 * P  # x columns/kb for the pair (blocks 4-5)
    ND = (n_full - 6) * P + (64 if packed else 0)  # blocks 6.. + packed tail
    out_rows = n_full * P + (64 if packed else QW if quarter else 0)

    nc = bacc.Bacc(None, target_bir_lowering=False, debug=False)
    # x is packed kb-major on the host so one DMA covers a GROUP of kb
    # blocks with >=2 KB per-partition rows (1 KB rows run ~280 GB/s),
    # and split by consuming unit so each piece can be kicked with wide
    # arrival margin (late-stream transfer latency is noisy by tens of
    # us under the per-queue FIFO + fair-share DMA engines).
    xA_d = nc.declare_dram_parameter("xA", [P, KB * NA], cdt, isOutput=False)
    xC_d = nc.declare_dram_parameter("xC", [P, KB * NC], cdt, isOutput=False)
    xD_d = nc.declare_dram_parameter("xD", [P, KB * ND], cdt, isOutput=False)
    w_d = nc.declare_dram_parameter("w", [D, D], cdt, isOutput=False)
    if quarter:
        xq_d = nc.declare_dram_parameter("xq", [P, KB * QW], cdt, isOutput=False)
        wq_d = nc.declare_dram_parameter("wq", [D, D], cdt, isOutput=False)
        wq_t = wq_d.rearrange("(k p) o -> p k o", p=P)
    out_d = nc.declare_dram_parameter("out", [out_rows, D], cdt, isOutput=True)

    w_t = w_d.rearrange("(k p) o -> p k o", p=P)  # [128, 16, 2048]

    with tile.TileContext(nc) as tc:
        with (
            tc.tile_pool(name="wp", bufs=1) as wp,
            tc.tile_pool(name="xp", bufs=1) as xp,
            # 24 staging tiles (3 MB): enough for quadA+quadB+pair slices so
            # a cast never waits on an out-DMA completing — out-kicks sit
            # behind ALL input kicks on Sync, which drain late on the run's
            # slowest core (op starvation -> cast stall -> PSUM-bank stall).
            tc.tile_pool(name="op", bufs=24) as op,
            tc.tile_pool(name="pp", bufs=8, space="PSUM") as pp,
        ):
            xa2 = [
                xp.tile([P, 2 * NA], cdt, tag=f"xa2_{j}", name=f"xa2_{j}")
                for j in range(KB // 2)
            ]
            wA = [
                wp.tile([P, HD], cdt, tag=f"wA{kb}", name=f"wA{kb}")
                for kb in range(KB)
            ]
            wB = [
                wp.tile([P, HD], cdt, tag=f"wB{kb}", name=f"wB{kb}")
                for kb in range(KB)
            ]

            # --- startup micro-kicks, alternating rings ---
            # Each in-flight transfer progresses at ~21 GB/s (fair share of
            # the 16 DMA engines), so the first tiles are split into small
            # parallel kicks in exact consumption order.
            x2 = 2 * NA
            early = [
                (nc.sync, xa2[0][:, :P], xA_d[:, :P]),
                (nc.scalar, wA[0][:, : NS // 2], w_t[:, 0, : NS // 2]),
                (nc.sync, wA[0][:, NS // 2 : NS], w_t[:, 0, NS // 2 : NS]),
                (nc.scalar, xa2[0][:, P : P + 192], xA_d[:, P : P + 192]),
                (nc.sync, xa2[0][:, P + 192 : NA], xA_d[:, P + 192 : NA]),
                (nc.scalar, wA[0][:, NS : NS + 256], w_t[:, 0, NS : NS + 256]),
                (nc.sync, wA[0][:, NS + 256 :], w_t[:, 0, NS + 256 : HD]),
                (nc.scalar, xa2[0][:, NA : NA + 256], xA_d[:, NA : NA + 256]),
                (nc.sync, xa2[0][:, NA + 256 :], xA_d[:, NA + 256 : x2]),
                (nc.scalar, wA[1][:, :NS], w_t[:, 1, :NS]),
                (nc.sync, wA[1][:, NS:], w_t[:, 1, NS:HD]),
                (nc.scalar, xa2[1][:, :NA], xA_d[:, x2 : x2 + NA]),
                (nc.sync, xa2[1][:, NA:], xA_d[:, x2 + NA : 2 * x2]),
                (nc.scalar, wA[2][:, :NS], w_t[:, 2, :NS]),
                (nc.sync, wA[2][:, NS:], w_t[:, 2, NS:HD]),
                (nc.scalar, wA[3][:, :NS], w_t[:, 3, :NS]),
                (nc.sync, wA[3][:, NS:], w_t[:, 3, NS:HD]),
            ]
            for ring, dst, src in early:
                ring.dma_start(dst, src)
            # bulk stream on Sync in consumption order
            for j in range(2, KB // 2):
                nc.sync.dma_start(xa2[j][:], xA_d[:, j * x2 : (j + 1) * x2])
                nc.sync.dma_start(wA[2 * j][:], w_t[:, 2 * j, :HD])
                nc.sync.dma_start(wA[2 * j + 1][:], w_t[:, 2 * j + 1, :HD])
            # pair-x (needed ~64us) rides right behind the wA stream and
            # ahead of most of wB; blocks-6+ x interleaves 1:1 with wB.
            # Everything late-stream gets >=4us (mostly >=30us) of margin.
            xc4 = [
                xp.tile([P, 4 * NC], cdt, tag=f"xc4_{j}", name=f"xc4_{j}")
                for j in range(KB // 4)
            ]
            xd4 = [
                xp.tile([P, 4 * ND], cdt, tag=f"xd4_{j}", name=f"xd4_{j}")
                for j in range(KB // 4)
            ]
            for j in range(KB // 4):
                nc.sync.dma_start(wB[2 * j][:], w_t[:, 2 * j, HD:])
                nc.sync.dma_start(
                    xc4[j][:], xC_d[:, j * 4 * NC : (j + 1) * 4 * NC]
                )
                nc.sync.dma_start(wB[2 * j + 1][:], w_t[:, 2 * j + 1, HD:])
            for j in range(KB // 4):
                nc.sync.dma_start(wB[KB // 2 + 2 * j][:], w_t[:, KB // 2 + 2 * j, HD:])
                nc.sync.dma_start(
                    xd4[j][:], xD_d[:, j * 4 * ND : (j + 1) * 4 * ND]
                )
                nc.sync.dma_start(
                    wB[KB // 2 + 2 * j + 1][:], w_t[:, KB // 2 + 2 * j + 1, HD:]
                )
            if quarter:
                xq_sb = xp.tile([P, KB * QW], cdt, tag="xq", name="xq")
                nc.sync.dma_start(xq_sb[:], xq_d[:])
                wq = []
                for kb in range(KB):
                    wk = wp.tile([P, D], cdt, tag=f"wq{kb}", name=f"wq{kb}")
                    nc.sync.dma_start(wk[:], wq_t[:, kb, :])
                    wq.append(wk)

            def w_slice(kb, osl):
                w = wA[kb] if osl < 2 else wB[kb]
                s = (osl % 2) * NS
                return w[:, s : s + NS]

            def lhs(kb, b):
                if b < 4:
                    off = (kb % 2) * NA + b * P
                    return xa2[kb // 2][:, off : off + P]
                if b < 6:
                    off = (kb % 4) * NC + (b - 4) * P
                    return xc4[kb // 4][:, off : off + P]
                off = (kb % 4) * ND + (b - 6) * P
                return xd4[kb // 4][:, off : off + P]

            def lhs_packed(kb):
                off = (kb % 4) * ND + (n_full - 6) * P
                return xd4[kb // 4][:, off : off + 64]

            # --- PE pre-warm: HAM clock-gates the PE at 1.2 GHz until
            # ~3.4us of sustained activity; dummy matmuls bridge first-data
            # DMA latency so real matmuls run warm.
            warm = xp.tile([P, 64], cdt, tag="warm")
            nc.vector.memset(warm[:], 0.0)
            pw = pp.tile([P, NS], mybir.dt.float32, tag="ps", name="ps_warm")
            # sized so real matmuls start ~12.5us with ~3 kb-groups of data
            # buffered and the clock already at 2.4 GHz — the DMA ramp can't
            # feed a 1.7us/kb warm pace before that, and an early data stall
            # resets the HAM activity window (cold cascade on slow cores)
            for _ in range(150):
                nc.tensor.matmul(
                    pw[:64, :64],
                    lhsT=warm[:, :64],
                    rhs=warm[:, :64],
                    start=True,
                    stop=True,
                )

            cast_idx = [0]

            def evac(slices, rings=(nc.sync,)):
                # psum -> sbuf (fp32 -> cdt cast) on alternating Vector /
                # Scalar queues — casts free PSUM banks for the next unit,
                # so those queues carry nothing else. Casts are emitted
                # before any out-kick; kicks ride the given ring(s), where
                # (in program order) they sit behind the input kicks and
                # drain once those finish.
                kicks = []
                for ps_ap, rows, r0, osl in slices:
                    o_sb = op.tile([P, NS], cdt, tag="o", name=f"o_{r0}_{osl}")
                    if cast_idx[0] % 2 == 0:
                        nc.vector.tensor_copy(o_sb[rows, :], ps_ap)
                    else:
                        nc.scalar.copy(o_sb[rows, :], ps_ap)
                    cast_idx[0] += 1
                    kicks.append((o_sb, rows, r0, osl))
                for i, (o_sb, rows, r0, osl) in enumerate(kicks):
                    nrows = (rows.stop if rows.stop is not None else P) - (
                        rows.start or 0
                    )
                    rings[i % len(rings)].dma_start(
                        out_d[r0 : r0 + nrows, osl * NS : (osl + 1) * NS],
                        o_sb[rows, :],
                    )

            full = slice(0, P)

            def run_group(blocks, osls):
                # kb-major accumulation over the given (block, os) banks
                ps = {
                    (b, osl): pp.tile(
                        [P, NS], mybir.dt.float32, tag="ps", name=f"ps_{b}_{osl}"
                    )
                    for b in blocks
                    for osl in osls
                }
                for kb in range(KB):
                    for b in blocks:
                        for osl in osls:
                            nc.tensor.matmul(
                                ps[(b, osl)][:],
                                lhsT=lhs(kb, b),
                                rhs=w_slice(kb, osl),
                                start=(kb == 0),
                                stop=(kb == KB - 1),
                            )
                evac(
                    [(ps[(b, o)][:], full, b * P, o) for b in blocks for o in osls]
                )

            def run_packed(os_pair, rings=(nc.sync,)):
                # 64-wide half-block: the two os slices run CONCURRENTLY in
                # the PE's column-group halves (tile_position auto-derived
                # from PSUM base partition 0/64; separate banks).
                ps = {
                    osl: pp.tile(
                        [P, NS], mybir.dt.float32, tag="ps", name=f"ps_pk{osl}"
                    )
                    for osl in os_pair
                }
                for kb in range(KB):
                    for osl in os_pair:
                        dst = ps[osl][:64, :] if osl % 2 == 0 else ps[osl][64:, :]
                        nc.tensor.matmul(
                            dst,
                            lhsT=lhs_packed(kb),
                            rhs=w_slice(kb, osl),
                            start=(kb == 0),
                            stop=(kb == KB - 1),
                        )
                slices = []
                for osl in os_pair:
                    rows = slice(0, 64) if osl % 2 == 0 else slice(64, P)
                    slices.append((ps[osl][rows, :], rows, n_full * P, osl))
                evac(slices, rings=rings)

            def run_bunit():
                # 32 overflow tokens of a second expert: all four os slices
                # run CONCURRENTLY in the PE's four 32-wide column groups
                # (explicit tile_position; one PSUM bank per group).
                ps = [
                    pp.tile([P, NS], mybir.dt.float32, tag="ps", name=f"ps_q{g}")
                    for g in range(4)
                ]
                for kb in range(KB):
                    for g in range(4):
                        nc.tensor.matmul(
                            ps[g][g * QW : (g + 1) * QW, :],
                            lhsT=xq_sb[:, kb * QW : (kb + 1) * QW],
                            rhs=wq[kb][:, g * NS : (g + 1) * NS],
                            start=(kb == 0),
                            stop=(kb == KB - 1),
                            tile_position=(0, g * QW),
                        )
                slices = []
                for g in range(4):
                    rows = slice(g * QW, (g + 1) * QW)
                    slices.append((ps[g][rows, :], rows, n_full * P, g))
                evac(slices, rings=(nc.sync, nc.scalar))

            # --- unit schedule ---
            run_group([0, 1, 2, 3], (0, 1))  # quadA: rides the startup stream
            run_group([0, 1, 2, 3], (2, 3))  # quadB
            mids = list(range(4, n_full - 2))
            while len(mids) >= 2:
                run_group(mids[:2], (0, 1, 2, 3))
                mids = mids[2:]
            if mids:
                run_group(mids, (0, 1, 2, 3))
            if packed:
                run_packed((0, 1))
            run_group([n_full - 2], (0, 1, 2, 3))
            run_group([n_full - 1], (0, 1, 2, 3))
            if packed:
                run_packed((2, 3), rings=(nc.sync, nc.scalar))
            if quarter:
                run_bunit()
    nc.compile()
    return nc


def _get_nc(n_full, tail, compute_dt):
    key = (n_full, tail, compute_dt)
    if key not in _cache:
        _cache[key] = _build(n_full, tail, compute_dt)
    return _cache[key]


def _pack_x(xt):
    # kb-major packs: xA[p, kb*NA + t] = xt[kb*128+p, t] for the quad's
    # columns (blocks 0-3), xC for blocks 4-5, xD for blocks 6.. + tail
    C = xt.shape[1]
    NA, NC = 4 * P, 2 * P
    x3 = xt.reshape(KB, P, C)

    def pack(lo, hi):
        return np.ascontiguousarray(
            x3[:, :, lo:hi].transpose(1, 0, 2).reshape(P, KB * (hi - lo))
        )

    return pack(0, NA), pack(NA, NA + NC), pack(NA + NC, C)


def kernel(tokens, weight, exp_ids, _trace=False, _compute_dt="float16"):
    _ensure_imports()
    from concourse.bass_utils import run_bass_kernel_spmd

    tokens = np.asarray(tokens)
    weight = np.asarray(weight)
    exp_ids = np.asarray(exp_ids)
    T = tokens.shape[0]

    order = np.argsort(exp_ids, kind="stable")
    counts = np.bincount(exp_ids, minlength=E)
    starts = np.zeros(E + 1, dtype=np.int64)
    np.cumsum(counts, out=starts[1:])

    npdt = _np_dt(_compute_dt)
    tokens_c = tokens.astype(npdt)
    weight_c = weight.astype(npdt)

    CAP_A = 8 * P  # per-core full-block capacity
    # overflow chunks: expert, start within expert's sorted run, length
    chunks = []
    for e in range(E):
        over = int(counts[e]) - CAP_A
        s = CAP_A
        while over > 0:
            n = min(QW, over)
            chunks.append((e, s, n))
            s += n
            over -= n

    if 0 < len(chunks) <= E:
        # quarter-rebalanced path: core e gets expert e's first <=1024
        # tokens as 8 full blocks; chunk i rides core i's B-unit.
        n_full, tail = 8, "quarter"
        NA = 4 * P
        in_maps = []
        for c in range(E):
            na = min(int(counts[c]), CAP_A)
            idx = order[starts[c] : starts[c] + na]
            xt = np.zeros((D, CAP_A), dtype=npdt)
            xt[:, :na] = tokens_c[idx].T
            xA, xC, xD = _pack_x(xt)
            if c < len(chunks):
                e, s, n = chunks[c]
                idx_b = order[starts[e] + s : starts[e] + s + n]
                xqt = np.zeros((D, QW), dtype=npdt)
                xqt[:, :n] = tokens_c[idx_b].T
                wq = np.ascontiguousarray(weight_c[e])
            else:
                xqt = np.zeros((D, QW), dtype=npdt)
                wq = np.ascontiguousarray(weight_c[c])  # unused; avoids zeros
            xq = np.ascontiguousarray(
                xqt.reshape(KB, P, QW).transpose(1, 0, 2).reshape(P, KB * QW)
            )
            in_maps.append(
                {
                    "xA": xA,
                    "xC": xC,
                    "xD": xD,
                    "w": np.ascontiguousarray(weight_c[c]),
                    "xq": xq,
                    "wq": wq,
                }
            )
    else:
        # fallback: pure expert-parallel with per-core capacity from the
        # largest expert (packed 64-token half-block when the remainder fits)
        cap = max(int(counts.max()), 6 * P + 1)
        n_full = cap // P
        rem = cap - n_full * P
        if rem > 64:
            n_full += 1
            tail = "none"
        else:
            tail = "packed" if rem > 0 else "none"
        CX = n_full * P + (64 if tail == "packed" else 0)
        NA = 4 * P
        in_maps = []
        for e in range(E):
            idx = order[starts[e] : starts[e + 1]]
            xt = np.zeros((D, CX), dtype=npdt)
            xt[:, : counts[e]] = tokens_c[idx].T
            xA, xC, xD = _pack_x(xt)
            in_maps.append(
                {"xA": xA, "xC": xC, "xD": xD, "w": np.ascontiguousarray(weight_c[e])}
            )

    nc = _get_nc(n_full, tail, _compute_dt)
    res = run_bass_kernel_spmd(
        nc,
        in_maps,
        core_ids=list(range(E)),
        trace=_trace,
        trace_cores=list(range(E)) if _trace else None,
    )

    out = np.empty((T, D), dtype=np.float32)
    if tail == "quarter":
        for c in range(E):
            na = min(int(counts[c]), CAP_A)
            idx = order[starts[c] : starts[c] + na]
            out[idx] = res.results[c]["out"][:na, :].astype(np.float32)
            if c < len(chunks):
                e, s, n = chunks[c]
                idx_b = order[starts[e] + s : starts[e] + s + n]
                out[idx_b] = res.results[c]["out"][CAP_A : CAP_A + n, :].astype(
                    np.float32
                )
    else:
        for e in range(E):
            idx = order[starts[e] : starts[e + 1]]
            out[idx] = res.results[e]["out"][: counts[e], :].astype(np.float32)
    if _trace:
        return out, res
    return out
